# revision 33
# baseline (speedup 1.0000x reference)
"""Trainium2 Bass kernel for nn_AutoregressiveDecoder.

Reference computation (B=2048, T=1024, D=32, S=2):
    s_{t+1} = s_t @ Ws.T + z_t @ Wz.T        (Ws = W[:, :2], Wz = W[:, 2:])
    out[:, t] = s_t,  s_0 = init_states      -> (B, T, S) fp32

Strategy: data-parallel over 8 cores (256 batch rows each). The sequential
scan is re-expressed as 16 chunks of C=64 steps; within a chunk all 64
outputs are produced by ONE PE accumulation group against a host-precomputed
block-Toeplitz matrix Q[(tl,d),(j,s)] = (Wz^T M^{j-1-tl})[d,s] (M = Ws^T),
plus a carry-in term. The inter-chunk carry is fused into the next chunk's
accumulation:  O_{k+1} += s_last(k)·(M R) + z_last(k)·(Wz^T R),  where
R[(s'),(j,s)] = M^j, so there is no separate carry matmul group at all.
The carry state is hi/lo-split in fp16 (scaled by 1/16 against overflow) so
the 16-step chain keeps fp32-level accuracy.

z is pre-transposed AND pre-converted to fp16 on the host (m10 ~ the PE's
native f32r m11 precision, half the HBM traffic), so the kernel needs no
on-chip transposes — all PE work is plain fp16 matmuls, which also keeps
the PE HAM clock-gate warm (transpose-mode ops do not count as PE activity
and previously kept the clock throttled at 1.2 GHz).

Output rows use REVERSED j order (row = (C-1-j)*S + s) so the last state of
a chunk lands on partitions 0:2 (partition bases must be 32-aligned).
"""

import numpy as np

B, T, D, S = 2048, 1024, 32, 2
C = 64                  # time steps per chunk
NCORES = 8
BL = B // NCORES        # 256 batch rows per core
KT = C * D // 128       # 16 K-tiles of 128 per chunk
CSC = 1.0 / 16.0        # carry scale (power of two; MR is stored x16)


# ---------------------------------------------------------------------------
# host-side helpers
# ---------------------------------------------------------------------------

def _f16(v):
    return np.asarray(v, np.float32).astype(np.float16)


def _host_constants(W):
    """Operand matrices from W (fp64 powers -> fp16).

    Output column index m = (C-1-j)*S + s  (reversed j)."""
    W64 = W.astype(np.float64)
    M = W64[:, :S].T            # (2, 2)
    WzT = W64[:, S:].T          # (32, 2)

    Mpow = [np.eye(S)]
    for _ in range(C + 1):
        Mpow.append(Mpow[-1] @ M)

    def col(j):
        return (C - 1 - j) * S

    Q = np.zeros((C * D, C * S), np.float64)
    R = np.zeros((S, C * S), np.float64)
    for j in range(C):
        R[:, col(j):col(j) + S] = Mpow[j]
        for tl in range(j):
            Q[tl * D:(tl + 1) * D, col(j):col(j) + S] = WzT @ Mpow[j - 1 - tl]

    MR = (M @ R) / CSC          # carry is stored scaled by CSC
    QU = WzT @ R

    Rf = np.float32(R)
    R_hi = _f16(Rf)
    R_lo = _f16(Rf - R_hi)
    MRf = np.float32(MR)
    MR_hi = _f16(MRf)
    MR_lo = _f16(MRf - MR_hi)

    # QU padded to K=128: rows 96:128 (t-local 63 of the last K-tile) = WzT R
    qu = np.zeros((128, C * S), np.float16)
    qu[96:128, :] = _f16(QU)

    # swizzled so each SBUF partition's data is one contiguous run
    qmat = np.ascontiguousarray(
        _f16(Q).reshape(KT, 128, C * S).transpose(1, 0, 2))  # (p, kt, m)

    return {
        "qmat": qmat,                                  # (128, 16, 128) fp16
        "rhi": np.ascontiguousarray(R_hi),             # (2, 128)
        "rlo": np.ascontiguousarray(R_lo),             # (2, 128)
        "mrhi": np.ascontiguousarray(MR_hi),           # (2, 128)
        "mrlo": np.ascontiguousarray(MR_lo),           # (2, 128)
        "qu": qu,                                      # (128, 128)
    }


def _host_z(z, nch):
    """Pre-transpose z to (core, chunk, p, kt, b) fp16 — swizzled so each
    SBUF partition's chunk data is one contiguous 8KB run."""
    zc = z[:, :nch * C, :]
    zt = _f16(zc).reshape(NCORES, BL, nch, KT, 128)    # (core, b, chunk, kt, p)
    zt = np.ascontiguousarray(zt.transpose(0, 2, 4, 3, 1))  # (core, chunk, p, kt, b)
    zt = zt.reshape(NCORES, nch // 2, 2, 128, KT * BL)
    zt = np.ascontiguousarray(zt.transpose(0, 1, 3, 2, 4))  # (core, pair, p, 2, ktb)
    return zt.reshape(NCORES, nch // 2, 128, 2 * KT * BL)


# ---------------------------------------------------------------------------
# workarounds for this container's walrus (max 1 sem-wait per instruction)
# ---------------------------------------------------------------------------

def _install_patches():
    import concourse.tile as tile
    import concourse.mybir as mybir
    from bass_rust import ScopedClock

    if getattr(tile.TileContext, "_ard_patched", False):
        return

    def _drain_and_barrier(self, tick_clock, wait_clock):
        nc = self.nc
        probe = nc.sync.nop(nofuse=True, hint="tail_wait_spread")
        wait_clock.add_sem_waits(
            probe.ins, ScopedClock({None: tick_clock.global_clock})
        )
        si = probe.ins.sync_info
        waits = list(si.on_wait) if si is not None else []
        updates = list(si.on_update) if si is not None else []
        if len(waits) > 1:
            probe.ins.sync_info = mybir.SyncInfo(on_wait=waits[:1], on_update=updates)
            for w in waits[1:]:
                n2 = nc.sync.nop(nofuse=True, hint="tail_wait_spread")
                n2.ins.sync_info = mybir.SyncInfo(on_wait=[w], on_update=[])
        nc.sync.drain()
        nc.all_engine_barrier()
        assert self.sems is not None
        popped = nc._tile_sem_poison_stack.pop()
        assert popped is self._sem_poison
        nc.clear_and_free_semaphores(list(self.sems.allocated().values()))

    tile.TileContext._drain_and_barrier = _drain_and_barrier
    tile.TileContext._ard_patched = True


def _spread_waits(nc):
    """Move excess sem-waits (>1) onto same-engine NoOps inserted just before
    the owning instruction (engines are in-order, so semantics hold)."""
    import concourse.mybir as mybir

    ctr = 0
    for f in nc.m.functions:
        for b in f.blocks:
            out = []
            changed = False
            for inst in b.instructions:
                si = inst.sync_info
                waits = list(si.on_wait) if si is not None else []
                if len(waits) > 1 and inst.engine != mybir.EngineType.Unassigned:
                    changed = True
                    for w in waits[:-1]:
                        ctr += 1
                        out.append(
                            mybir.InstNoOp(
                                name=f"waitspread-{ctr}",
                                sync_info=mybir.SyncInfo(on_wait=[w], on_update=[]),
                                bass_nofuse=True,
                                engine=inst.engine,
                            )
                        )
                    inst.sync_info = mybir.SyncInfo(
                        on_wait=waits[-1:], on_update=list(si.on_update)
                    )
                out.append(inst)
            if changed:
                b.instructions = out
    return ctr


# ---------------------------------------------------------------------------
# device program
# ---------------------------------------------------------------------------

def _build_nc(nch):
    import concourse.bass as bass
    import concourse.tile as tile
    import concourse.mybir as mybir

    _install_patches()
    f16 = mybir.dt.float16
    f32 = mybir.dt.float32
    PSUM = bass.MemorySpace.PSUM
    AluOp = mybir.AluOpType

    nc = bass.Bass(trn_type="TRN2", target_bir_lowering=False, debug=False)
    zin = nc.dram_tensor("zin", [nch // 2, 128, 2 * KT * BL], f16, kind="ExternalInput")
    qmat = nc.dram_tensor("qmat", [128, KT * C * S], f16, kind="ExternalInput")
    rhi = nc.dram_tensor("rhi", [2, C * S], f16, kind="ExternalInput")
    rlo = nc.dram_tensor("rlo", [2, C * S], f16, kind="ExternalInput")
    mrhi = nc.dram_tensor("mrhi", [2, C * S], f16, kind="ExternalInput")
    mrlo = nc.dram_tensor("mrlo", [2, C * S], f16, kind="ExternalInput")
    qu = nc.dram_tensor("qu", [128, C * S], f16, kind="ExternalInput")
    c0hi = nc.dram_tensor("c0hi", [2, BL], f16, kind="ExternalInput")
    c0lo = nc.dram_tensor("c0lo", [2, BL], f16, kind="ExternalInput")
    out = nc.dram_tensor("out", [128, nch * BL], f32, kind="ExternalOutput")

    with tile.TileContext(nc) as tc:
        with (
            tc.tile_pool(name="const", bufs=1) as const,
            tc.tile_pool(name="zbuf", bufs=6) as zbuf,
            tc.tile_pool(name="obuf", bufs=2) as obuf,
            tc.tile_pool(name="cbuf", bufs=3) as cbuf,
            tc.tile_pool(name="outps", bufs=4, space=PSUM) as outps,
        ):
            # constants go through the ACT HWDGE ring so they stream in
            # parallel with the z loads on the SP ring
            qparts = []
            for q4 in range(4):
                w = KT * C * S // 4
                qp = const.tile([128, w], f16, tag=f"q{q4}", name=f"q{q4}")
                nc.scalar.dma_start(qp[:], qmat.ap()[:, q4 * w:(q4 + 1) * w])
                qparts.append(qp)
            rhisb = const.tile([2, C * S], f16)
            nc.scalar.dma_start(rhisb[:], rhi.ap())
            rlosb = const.tile([2, C * S], f16)
            nc.scalar.dma_start(rlosb[:], rlo.ap())
            mrhisb = const.tile([2, C * S], f16)
            nc.scalar.dma_start(mrhisb[:], mrhi.ap())
            mrlosb = const.tile([2, C * S], f16)
            nc.scalar.dma_start(mrlosb[:], mrlo.ap())
            qusb = const.tile([128, C * S], f16)
            nc.scalar.dma_start(qusb[:], qu.ap())
            chi0 = const.tile([2, BL], f16)
            nc.scalar.dma_start(chi0[:], c0hi.ap())
            clo0 = const.tile([2, BL], f16)
            nc.scalar.dma_start(clo0[:], c0lo.ap())

            def dma_z(p):
                zt = zbuf.tile([128, 2 * KT * BL], f16, tag="z", name="zt")
                nc.sync.dma_start(zt[:], zin.ap()[p])
                return [(zt, 0)]

            def dma_z_parts(p):
                parts = []
                w = 2 * KT * BL // 4
                for h in range(4):
                    zp = zbuf.tile([128, w], f16, tag=f"zp{h}", name=f"zp{h}", bufs=1)
                    nc.sync.dma_start(zp[:], zin.ap()[p][:, h * w:(h + 1) * w])
                    parts.append((zp, h * w))
                return parts

            NPAIR = nch // 2
            zs = {p: (dma_z_parts(p) if p < 1 else dma_z(p))
                  for p in range(min(NPAIR, 3))}

            def zslice(zk, lo, width):
                for tile_, off in reversed(zk):
                    if lo >= off:
                        return tile_[:, lo - off:lo - off + width]
                raise AssertionError

            obs = {}
            shilo = {}
            for k in range(nch):
                if k % 2 == 0 and k // 2 + 3 < NPAIR:
                    zs[k // 2 + 3] = dma_z(k // 2 + 3)
                zk = zs[k // 2]
                zoff = (k % 2) * KT * BL

                pout = outps.tile([128, BL], f32, tag="pout", name="pout")
                for kt in range(KT):
                    qp = qparts[kt // 4]
                    nc.tensor.matmul(
                        pout[:], qp[:, (kt % 4) * 128:(kt % 4) * 128 + 128],
                        zslice(zk, zoff + kt * BL, BL),
                        start=(kt == 0), stop=False,
                    )
                # carry-add (fused: previous chunk's last state + last z step)
                if k == 0:
                    nc.tensor.matmul(pout[:], rhisb[:], chi0[:], start=False, stop=False)
                    nc.tensor.matmul(pout[:], rhisb[:], clo0[:], start=False, stop=False)
                    nc.tensor.matmul(pout[:], rlosb[:], chi0[:], start=False, stop=True)
                else:
                    shi, slo = shilo[k - 1]
                    zprev = zs[(k - 1) // 2]
                    zpoff = ((k - 1) % 2) * KT * BL
                    nc.tensor.matmul(pout[:], mrhisb[:], shi[:], start=False, stop=False)
                    nc.tensor.matmul(pout[:], mrhisb[:], slo[:], start=False, stop=False)
                    nc.tensor.matmul(pout[:], mrlosb[:], shi[:], start=False, stop=False)
                    nc.tensor.matmul(
                        pout[:], qusb[:], zslice(zprev, zpoff + (KT - 1) * BL, BL),
                        start=False, stop=True,
                    )
                    if k % 2 == 0 and k >= 2:
                        del zs[k // 2 - 1]

                # carry state for the next chunk: scaled hi/lo split of the
                # last state (psum rows 0:2, thanks to reversed j order)
                if k < nch - 1:
                    shi = cbuf.tile([2, BL], f16, tag="shi", name="shi")
                    nc.scalar.mul(shi[:], pout[0:2, :], CSC)
                    slo = cbuf.tile([2, BL], f16, tag="slo", name="slo")
                    nc.vector.scalar_tensor_tensor(
                        slo[:], pout[0:2, :], CSC, shi[:],
                        op0=AluOp.mult, op1=AluOp.subtract,
                    )
                    shilo[k] = (shi, slo)

                # stage + write out every 2 chunks
                if k % 2 == 0:
                    obs[k // 2] = obuf.tile([128, 2 * BL], f32, tag="ob", name="ob")
                ob = obs[k // 2]
                nc.vector.tensor_copy(ob[:, (k % 2) * BL:(k % 2) * BL + BL], pout[:])
                if k % 2 == 1:
                    g = k // 2
                    nc.scalar.dma_start(
                        out.ap()[:, g * 2 * BL:(g + 1) * 2 * BL], ob[:]
                    )

    _spread_waits(nc)
    return nc


_CACHE = {}


def _get_nc(nch):
    if nch not in _CACHE:
        _CACHE[nch] = _build_nc(nch)
    return _CACHE[nch]


# ---------------------------------------------------------------------------
# entry point
# ---------------------------------------------------------------------------

def _run(init_states, z, W, nch, core_ids, trace=False):
    from concourse.bass_utils import run_bass_kernel_spmd

    consts = _host_constants(W)
    zt = _host_z(np.asarray(z), nch)
    ncores = len(core_ids)
    in_maps = []
    for i in range(ncores):
        sl = slice(i * BL, (i + 1) * BL)
        init_T = np.ascontiguousarray(init_states[sl].T, np.float32)  # (2, BL)
        hi = _f16(init_T)
        lo = _f16(init_T - hi)
        in_maps.append({
            "zin": zt[i],
            "qmat": consts["qmat"],
            "rhi": consts["rhi"],
            "rlo": consts["rlo"],
            "mrhi": consts["mrhi"],
            "mrlo": consts["mrlo"],
            "qu": consts["qu"],
            "c0hi": hi,
            "c0lo": lo,
        })

    nc = _get_nc(nch)
    kwargs = {}
    if trace:
        kwargs = dict(trace=True, trace_cores=list(core_ids))
    res = run_bass_kernel_spmd(nc, in_maps, core_ids=list(core_ids), **kwargs)

    outs = []
    for i in range(ncores):
        o = res.results[i]["out"]                       # (128, nch*BL)
        o = o.reshape(C, S, nch, BL)                    # (rev_j, s, k, b)
        o = o[::-1]                                     # undo reversed j
        o = np.transpose(o, (3, 2, 0, 1)).reshape(BL, nch * C, S)
        outs.append(o)
    full = np.concatenate(outs, axis=0).astype(np.float32)
    return full, res


def kernel(init_states, z, W):
    init_states = np.asarray(init_states, np.float32)
    z = np.asarray(z, np.float32)
    W = np.asarray(W, np.float32)
    full, _ = _run(init_states, z, W, T // C, list(range(NCORES)))
    return full


# revision 34
# speedup vs baseline: 1.0518x; 1.0518x over previous
"""Trainium2 Bass kernel for nn_AutoregressiveDecoder.

Reference computation (B=2048, T=1024, D=32, S=2):
    s_{t+1} = s_t @ Ws.T + z_t @ Wz.T        (Ws = W[:, :2], Wz = W[:, 2:])
    out[:, t] = s_t,  s_0 = init_states      -> (B, T, S) fp32

Strategy: data-parallel over 8 cores (256 batch rows each). The sequential
scan is re-expressed as 16 chunks of C=64 steps; within a chunk all 64
outputs are produced by ONE PE accumulation group against a host-precomputed
block-Toeplitz matrix Q[(tl,d),(j,s)] = (Wz^T M^{j-1-tl})[d,s] (M = Ws^T),
plus a carry-in term. The inter-chunk carry is fused into the next chunk's
accumulation:  O_{k+1} += s_last(k)·(M R) + z_last(k)·(Wz^T R),  where
R[(s'),(j,s)] = M^j, so there is no separate carry matmul group at all.
The carry state is hi/lo-split in fp16 (scaled by 1/16 against overflow) so
the 16-step chain keeps fp32-level accuracy.

z is pre-transposed AND pre-converted to fp16 on the host (m10 ~ the PE's
native f32r m11 precision, half the HBM traffic), so the kernel needs no
on-chip transposes — all PE work is plain fp16 matmuls, which also keeps
the PE HAM clock-gate warm (transpose-mode ops do not count as PE activity
and previously kept the clock throttled at 1.2 GHz).

Output rows use REVERSED j order (row = (C-1-j)*S + s) so the last state of
a chunk lands on partitions 0:2 (partition bases must be 32-aligned).
"""

import numpy as np

B, T, D, S = 2048, 1024, 32, 2
C = 64                  # time steps per chunk
NCORES = 8
BL = B // NCORES        # 256 batch rows per core
KT = C * D // 128       # 16 K-tiles of 128 per chunk
CSC = 1.0 / 16.0        # carry scale (power of two; MR is stored x16)


# ---------------------------------------------------------------------------
# host-side helpers
# ---------------------------------------------------------------------------

def _f16(v):
    return np.asarray(v, np.float32).astype(np.float16)


def _host_constants(W):
    """Operand matrices from W (fp64 powers -> fp16).

    Output column index m = (C-1-j)*S + s  (reversed j)."""
    W64 = W.astype(np.float64)
    M = W64[:, :S].T            # (2, 2)
    WzT = W64[:, S:].T          # (32, 2)

    Mpow = [np.eye(S)]
    for _ in range(C + 1):
        Mpow.append(Mpow[-1] @ M)

    def col(j):
        return (C - 1 - j) * S

    Q = np.zeros((C * D, C * S), np.float64)
    R = np.zeros((S, C * S), np.float64)
    for j in range(C):
        R[:, col(j):col(j) + S] = Mpow[j]
        for tl in range(j):
            Q[tl * D:(tl + 1) * D, col(j):col(j) + S] = WzT @ Mpow[j - 1 - tl]

    MR = (M @ R) / CSC          # carry is stored scaled by CSC
    QU = WzT @ R

    Rf = np.float32(R)
    R_hi = _f16(Rf)
    R_lo = _f16(Rf - R_hi)
    MRf = np.float32(MR)
    MR_hi = _f16(MRf)
    MR_lo = _f16(MRf - MR_hi)

    # QU padded to K=128: rows 96:128 (t-local 63 of the last K-tile) = WzT R
    qu = np.zeros((128, C * S), np.float16)
    qu[96:128, :] = _f16(QU)

    # swizzled so each SBUF partition's data is one contiguous run
    qmat = np.ascontiguousarray(
        _f16(Q).reshape(KT, 128, C * S).transpose(1, 0, 2))  # (p, kt, m)

    return {
        "qmat": qmat,                                  # (128, 16, 128) fp16
        "rhi": np.ascontiguousarray(R_hi),             # (2, 128)
        "rlo": np.ascontiguousarray(R_lo),             # (2, 128)
        "mrhi": np.ascontiguousarray(MR_hi),           # (2, 128)
        "mrlo": np.ascontiguousarray(MR_lo),           # (2, 128)
        "qu": qu,                                      # (128, 128)
    }


def _host_z(z, nch):
    """Pre-transpose z to (core, chunk, p, kt, b) fp16 — swizzled so each
    SBUF partition's chunk data is one contiguous 8KB run."""
    zc = z[:, :nch * C, :]
    zt = _f16(zc).reshape(NCORES, BL, nch, KT, 128)    # (core, b, chunk, kt, p)
    zt = np.ascontiguousarray(zt.transpose(0, 2, 4, 3, 1))  # (core, chunk, p, kt, b)
    zt = zt.reshape(NCORES, nch // 2, 2, 128, KT * BL)
    zt = np.ascontiguousarray(zt.transpose(0, 1, 3, 2, 4))  # (core, pair, p, 2, ktb)
    return zt.reshape(NCORES, nch // 2, 128, 2 * KT * BL)


# ---------------------------------------------------------------------------
# workarounds for this container's walrus (max 1 sem-wait per instruction)
# ---------------------------------------------------------------------------

def _install_patches():
    import concourse.tile as tile
    import concourse.mybir as mybir
    from bass_rust import ScopedClock

    if getattr(tile.TileContext, "_ard_patched", False):
        return

    def _drain_and_barrier(self, tick_clock, wait_clock):
        nc = self.nc
        probe = nc.sync.nop(nofuse=True, hint="tail_wait_spread")
        wait_clock.add_sem_waits(
            probe.ins, ScopedClock({None: tick_clock.global_clock})
        )
        si = probe.ins.sync_info
        waits = list(si.on_wait) if si is not None else []
        updates = list(si.on_update) if si is not None else []
        if len(waits) > 1:
            probe.ins.sync_info = mybir.SyncInfo(on_wait=waits[:1], on_update=updates)
            for w in waits[1:]:
                n2 = nc.sync.nop(nofuse=True, hint="tail_wait_spread")
                n2.ins.sync_info = mybir.SyncInfo(on_wait=[w], on_update=[])
        nc.sync.drain()
        nc.all_engine_barrier()
        assert self.sems is not None
        popped = nc._tile_sem_poison_stack.pop()
        assert popped is self._sem_poison
        nc.clear_and_free_semaphores(list(self.sems.allocated().values()))

    tile.TileContext._drain_and_barrier = _drain_and_barrier
    tile.TileContext._ard_patched = True


def _spread_waits(nc):
    """Move excess sem-waits (>1) onto same-engine NoOps inserted just before
    the owning instruction (engines are in-order, so semantics hold)."""
    import concourse.mybir as mybir

    ctr = 0
    for f in nc.m.functions:
        for b in f.blocks:
            out = []
            changed = False
            for inst in b.instructions:
                si = inst.sync_info
                waits = list(si.on_wait) if si is not None else []
                if len(waits) > 1 and inst.engine != mybir.EngineType.Unassigned:
                    changed = True
                    for w in waits[:-1]:
                        ctr += 1
                        out.append(
                            mybir.InstNoOp(
                                name=f"waitspread-{ctr}",
                                sync_info=mybir.SyncInfo(on_wait=[w], on_update=[]),
                                bass_nofuse=True,
                                engine=inst.engine,
                            )
                        )
                    inst.sync_info = mybir.SyncInfo(
                        on_wait=waits[-1:], on_update=list(si.on_update)
                    )
                out.append(inst)
            if changed:
                b.instructions = out
    return ctr


# ---------------------------------------------------------------------------
# device program
# ---------------------------------------------------------------------------

def _build_nc(nch):
    import concourse.bass as bass
    import concourse.tile as tile
    import concourse.mybir as mybir

    _install_patches()
    f16 = mybir.dt.float16
    f32 = mybir.dt.float32
    PSUM = bass.MemorySpace.PSUM
    AluOp = mybir.AluOpType

    nc = bass.Bass(trn_type="TRN2", target_bir_lowering=False, debug=False)
    zin = nc.dram_tensor("zin", [nch // 2, 128, 2 * KT * BL], f16, kind="ExternalInput")
    qmat = nc.dram_tensor("qmat", [128, KT * C * S], f16, kind="ExternalInput")
    rhi = nc.dram_tensor("rhi", [2, C * S], f16, kind="ExternalInput")
    rlo = nc.dram_tensor("rlo", [2, C * S], f16, kind="ExternalInput")
    mrhi = nc.dram_tensor("mrhi", [2, C * S], f16, kind="ExternalInput")
    mrlo = nc.dram_tensor("mrlo", [2, C * S], f16, kind="ExternalInput")
    qu = nc.dram_tensor("qu", [128, C * S], f16, kind="ExternalInput")
    c0hi = nc.dram_tensor("c0hi", [2, BL], f16, kind="ExternalInput")
    c0lo = nc.dram_tensor("c0lo", [2, BL], f16, kind="ExternalInput")
    out = nc.dram_tensor("out", [128, nch * BL], f32, kind="ExternalOutput")

    with tile.TileContext(nc) as tc:
        with (
            tc.tile_pool(name="const", bufs=1) as const,
            tc.tile_pool(name="zbuf", bufs=6) as zbuf,
            tc.tile_pool(name="obuf", bufs=2) as obuf,
            tc.tile_pool(name="cbuf", bufs=3) as cbuf,
            tc.tile_pool(name="outps", bufs=4, space=PSUM) as outps,
        ):
            # constants go through the ACT HWDGE ring so they stream in
            # parallel with the z loads on the SP ring
            qparts = []
            for q4 in range(4):
                w = KT * C * S // 4
                qp = const.tile([128, w], f16, tag=f"q{q4}", name=f"q{q4}")
                nc.scalar.dma_start(qp[:], qmat.ap()[:, q4 * w:(q4 + 1) * w])
                qparts.append(qp)
            rhisb = const.tile([2, C * S], f16)
            nc.scalar.dma_start(rhisb[:], rhi.ap())
            rlosb = const.tile([2, C * S], f16)
            nc.scalar.dma_start(rlosb[:], rlo.ap())
            mrhisb = const.tile([2, C * S], f16)
            nc.scalar.dma_start(mrhisb[:], mrhi.ap())
            mrlosb = const.tile([2, C * S], f16)
            nc.scalar.dma_start(mrlosb[:], mrlo.ap())
            qusb = const.tile([128, C * S], f16)
            nc.scalar.dma_start(qusb[:], qu.ap())
            chi0 = const.tile([2, BL], f16)
            nc.scalar.dma_start(chi0[:], c0hi.ap())
            clo0 = const.tile([2, BL], f16)
            nc.scalar.dma_start(clo0[:], c0lo.ap())

            def dma_z(p):
                zt = zbuf.tile([128, 2 * KT * BL], f16, tag="z", name="zt")
                nc.sync.dma_start(zt[:], zin.ap()[p])
                return [(zt, 0)]

            def dma_z_parts(p):
                parts = []
                w = 2 * KT * BL // 4
                for h in range(4):
                    zp = zbuf.tile([128, w], f16, tag=f"zp{h}", name=f"zp{h}", bufs=2)
                    nc.sync.dma_start(zp[:], zin.ap()[p][:, h * w:(h + 1) * w])
                    parts.append((zp, h * w))
                return parts

            NPAIR = nch // 2
            zs = {p: (dma_z_parts(p) if p < 2 else dma_z(p))
                  for p in range(min(NPAIR, 4))}

            def zslice(zk, lo, width):
                for tile_, off in reversed(zk):
                    if lo >= off:
                        return tile_[:, lo - off:lo - off + width]
                raise AssertionError

            obs = {}
            shilo = {}
            for k in range(nch):
                if k % 2 == 0 and k // 2 + 4 < NPAIR:
                    zs[k // 2 + 4] = dma_z(k // 2 + 4)
                zk = zs[k // 2]
                zoff = (k % 2) * KT * BL

                pout = outps.tile([128, BL], f32, tag="pout", name="pout")
                for kt in range(KT):
                    qp = qparts[kt // 4]
                    nc.tensor.matmul(
                        pout[:], qp[:, (kt % 4) * 128:(kt % 4) * 128 + 128],
                        zslice(zk, zoff + kt * BL, BL),
                        start=(kt == 0), stop=False,
                    )
                # carry-add (fused: previous chunk's last state + last z step)
                if k == 0:
                    nc.tensor.matmul(pout[:], rhisb[:], chi0[:], start=False, stop=False)
                    nc.tensor.matmul(pout[:], rhisb[:], clo0[:], start=False, stop=False)
                    nc.tensor.matmul(pout[:], rlosb[:], chi0[:], start=False, stop=True)
                else:
                    shi, slo = shilo[k - 1]
                    zprev = zs[(k - 1) // 2]
                    zpoff = ((k - 1) % 2) * KT * BL
                    nc.tensor.matmul(pout[:], mrhisb[:], shi[:], start=False, stop=False)
                    nc.tensor.matmul(pout[:], mrhisb[:], slo[:], start=False, stop=False)
                    nc.tensor.matmul(pout[:], mrlosb[:], shi[:], start=False, stop=False)
                    nc.tensor.matmul(
                        pout[:], qusb[:], zslice(zprev, zpoff + (KT - 1) * BL, BL),
                        start=False, stop=True,
                    )
                    if k % 2 == 0 and k >= 2:
                        del zs[k // 2 - 1]

                # carry state for the next chunk: scaled hi/lo split of the
                # last state (psum rows 0:2, thanks to reversed j order)
                if k < nch - 1:
                    shi = cbuf.tile([2, BL], f16, tag="shi", name="shi")
                    nc.scalar.mul(shi[:], pout[0:2, :], CSC)
                    slo = cbuf.tile([2, BL], f16, tag="slo", name="slo")
                    nc.vector.scalar_tensor_tensor(
                        slo[:], pout[0:2, :], CSC, shi[:],
                        op0=AluOp.mult, op1=AluOp.subtract,
                    )
                    shilo[k] = (shi, slo)

                # stage + write out every 2 chunks
                if k % 2 == 0:
                    obs[k // 2] = obuf.tile([128, 2 * BL], f32, tag="ob", name="ob")
                ob = obs[k // 2]
                nc.vector.tensor_copy(ob[:, (k % 2) * BL:(k % 2) * BL + BL], pout[:])
                if k % 2 == 1:
                    g = k // 2
                    nc.scalar.dma_start(
                        out.ap()[:, g * 2 * BL:(g + 1) * 2 * BL], ob[:]
                    )

    _spread_waits(nc)
    return nc


_CACHE = {}


def _get_nc(nch):
    if nch not in _CACHE:
        _CACHE[nch] = _build_nc(nch)
    return _CACHE[nch]


# ---------------------------------------------------------------------------
# entry point
# ---------------------------------------------------------------------------

def _run(init_states, z, W, nch, core_ids, trace=False):
    from concourse.bass_utils import run_bass_kernel_spmd

    consts = _host_constants(W)
    zt = _host_z(np.asarray(z), nch)
    ncores = len(core_ids)
    in_maps = []
    for i in range(ncores):
        sl = slice(i * BL, (i + 1) * BL)
        init_T = np.ascontiguousarray(init_states[sl].T, np.float32)  # (2, BL)
        hi = _f16(init_T)
        lo = _f16(init_T - hi)
        in_maps.append({
            "zin": zt[i],
            "qmat": consts["qmat"],
            "rhi": consts["rhi"],
            "rlo": consts["rlo"],
            "mrhi": consts["mrhi"],
            "mrlo": consts["mrlo"],
            "qu": consts["qu"],
            "c0hi": hi,
            "c0lo": lo,
        })

    nc = _get_nc(nch)
    kwargs = {}
    if trace:
        kwargs = dict(trace=True, trace_cores=list(core_ids))
    res = run_bass_kernel_spmd(nc, in_maps, core_ids=list(core_ids), **kwargs)

    outs = []
    for i in range(ncores):
        o = res.results[i]["out"]                       # (128, nch*BL)
        o = o.reshape(C, S, nch, BL)                    # (rev_j, s, k, b)
        o = o[::-1]                                     # undo reversed j
        o = np.transpose(o, (3, 2, 0, 1)).reshape(BL, nch * C, S)
        outs.append(o)
    full = np.concatenate(outs, axis=0).astype(np.float32)
    return full, res


def kernel(init_states, z, W):
    init_states = np.asarray(init_states, np.float32)
    z = np.asarray(z, np.float32)
    W = np.asarray(W, np.float32)
    full, _ = _run(init_states, z, W, T // C, list(range(NCORES)))
    return full
